# revision 30
# baseline (speedup 1.0000x reference)
"""Trainium2 Bass kernel for nn_AttentionGenerator (gnn_message_passing).

Reference math:
    f = einsum('oc,bctv->botv', Wf, feat) + bf          # 1x1 conv, Cout=64
    s_i = einsum('c,bctv->btv', Wa[:64], f)
    s_j = einsum('c,bctv->btv', Wa[64:], f)
    score[b,t,i,j] = s_i[b,t,i] + s_j[b,t,j] + ba
    atten = (exp(leaky_relu(score)) * A) / row_sum

Because f only enters through the two dot products, fold Wf/bf/Wa/ba on
the host into u1 = w1@Wf, u2 = w2@Wf (length-256 vectors) and the scalar
c0 = (w1+w2)@bf + ba.

Device pipeline (memory-bound problem -> minimize HBM bytes):
  * feat is sent in fp8 e4m3 (halves the dominant HBM stream vs bf16);
    u1/u2 are scaled by 64 into the fp8 normal range, with the 1/64
    descale folded into the exp input scale (max/mult commute with
    positive scaling, so leaky-relu can run on the scaled scores).
  * TensorE pass 1: per batch-pair, 18 DoubleRow matmuls (2 fp8
    contraction rows/cycle) contract the 256 channels for each graph
    node v -> sT[(v,o), t] in PSUM.  The 36 stationary columns are
    u1/u2 shifted per-v, so s_i and s_j come from a single feat pass.
  * ACT evacuates sT to a staged SBUF tile (bf16) whose row 36 is a
    constant ones row.
  * TensorE pass 2: score[t, (i,j)] = s1[t,i]+s2[t,j]+c0 is linear in
    the augmented sT, so one tiny matmul per t-block against a constant
    matrix G [37, 324] (0/1 pattern + c0 row) assembles the full biased
    score tile - no transposes, no DMA scatters, no broadcast-add.
  * exp(leaky(x)) = max(exp(x), exp(.1x)), and exp(.1x/64) with
    |arg|<=.07 is the linear Taylor 1+.1x/64 to ~2e-3: e1 = Exp on ACT
    runs in parallel with the e2 Taylor on DVE (single Exp table,
    loaded once - table switches cost 1.5us), then a packed-bf16 max.
  * gpsimd: *A; DVE: row-sum + reciprocal; normalize alternates
    DVE/gpsimd per t-block so both halves finish in parallel.
  * Junk matmuls on iota data during the initial feat DMA warm the PE
    HAM clock gate (cold PE runs at 1.2 GHz, warm 2.4; the monitor
    watches actual bit activity, so the data must be nonzero).
  * A guard DMA gates pair1's feat behind pair0 (the HWDGE rings
    round-robin all queued DMAs, which would starve pair0 otherwise).
  * Outputs are written bf16 and upcast on the host (atten in [0,1]).

Sharding: pure data parallel - batch B=32 split across 8 NeuronCores
(4 batches each), tiny params replicated, no cross-core comms.
"""

import json
import numpy as np
from contextlib import ExitStack

B, Cin, T, V = 32, 256, 256, 18
NCORES = 8
BPC = B // NCORES  # batches per core
NPAIR = BPC // 2  # batch pairs per core
SC = 64.0  # weight prescale so u1/u2 land in fp8-normal range
VV = V * V  # 324
WPAD = 80  # padded weight column pitch (DoubleRow k-tile step must be %16)

_cached_nc = None
_warmed = False


def _legalize_waits_json(bir_json):
    """Split instructions carrying >1 sync wait into single-wait NoOps plus
    the original instruction.  The walrus build in this container accepts at
    most ONE sync-wait command per instruction struct; concourse's Tile
    scheduler freely attaches several.  Hoisting the extra waits onto NoOps
    immediately before the instruction (same engine stream, same position)
    preserves semantics exactly - engines execute their stream in order."""
    bir = json.loads(bir_json)
    ctr = 0
    for fn in bir.get("functions", []):
        for blk in fn.get("blocks", []):
            insts = blk.get("instructions")
            if not insts:
                continue
            out = []
            for inst in insts:
                si = inst.get("sync_info") or {}
                waits = si.get("on_wait") or []
                if len(waits) > 1:
                    for w in waits[:-1]:
                        out.append(
                            {
                                "engine": inst.get("engine"),
                                "ins": [],
                                "name": f"wsplit-{ctr}",
                                "opcode": "NoOp",
                                "outs": [],
                                "sync_info": {"on_update": [], "on_wait": [w]},
                            }
                        )
                        ctr += 1
                    si = dict(si)
                    si["on_wait"] = [waits[-1]]
                    inst = dict(inst)
                    inst["sync_info"] = si
                out.append(inst)
            blk["instructions"] = out
    return json.dumps(bir).encode()


_wait_patch_done = False


def _install_wait_legalizer():
    global _wait_patch_done
    if _wait_patch_done:
        return
    import concourse.bass_utils as bass_utils
    import concourse.bass2jax as bass2jax

    orig = bass_utils.compile_bir_kernel

    def wrapped(bir_json, tmpdir, neff_name="file.neff"):
        return orig(_legalize_waits_json(bir_json), tmpdir, neff_name)

    bass_utils.compile_bir_kernel = wrapped
    bass2jax.compile_bir_kernel = wrapped
    _wait_patch_done = True


def _build_nc():
    import concourse.bass as bass
    import concourse.mybir as mybir
    import concourse.tile as tile
    from concourse.alu_op_type import AluOpType

    f32 = mybir.dt.float32
    bf16 = mybir.dt.bfloat16
    fp8 = mybir.dt.float8e4
    nc = bass.Bass(num_swdge_queues=4)

    # feat[pair, p, v, kt, (b2 t)]: channel c = kt*128 + p, fp8 e4m3
    feat = nc.dram_tensor(
        "feat", [NPAIR, 128, V, 2, 2 * T], fp8, kind="ExternalInput"
    )
    # wmat[p, kt, col]: zeros except col 36 = 64*u1[kt*128+p], col 37 = 64*u2
    wmat = nc.dram_tensor("wmat", [128, 2, WPAD], fp8, kind="ExternalInput")
    # amat[p, (tb, i, j)]: A duplicated per t-block, pre-replicated across
    # partitions on the host (a partition-broadcast DMA hammers one DRAM
    # page and steals SDMA bandwidth from the feat stream for ~10us)
    amat = nc.dram_tensor("amat", [128, 2 * VV], bf16, kind="ExternalInput")
    # gmat[(v,o) + ones-row, (i,j)]: score-assembly 0/1 matrix with the
    # bias row SC*c0 last - score = sT_aug.T @ G lands fully biased
    gmat = nc.dram_tensor("gmat", [2 * V + 1, VV], bf16, kind="ExternalInput")
    out = nc.dram_tensor("out", [BPC, 128, 2 * VV], bf16, kind="ExternalOutput")

    with ExitStack() as ctx:
        tc = ctx.enter_context(tile.TileContext(nc))
        singles = ctx.enter_context(tc.tile_pool(name="singles", bufs=1))
        fpool = ctx.enter_context(tc.tile_pool(name="fpool", bufs=NPAIR))
        ps_sT = ctx.enter_context(tc.tile_pool(name="ps_sT", bufs=2, space="PSUM"))
        ps_sc = ctx.enter_context(tc.tile_pool(name="ps_sc", bufs=2, space="PSUM"))
        ps_junk = ctx.enter_context(tc.tile_pool(name="ps_junk", bufs=2, space="PSUM"))
        work = ctx.enter_context(tc.tile_pool(name="work", bufs=2))
        opool = ctx.enter_context(tc.tile_pool(name="opool", bufs=2))

        # feat loads first (highest tile priority): 2 per pair (v 0:9 /
        # 9:18) on the SP ring.  The HWDGE rings round-robin ALL queued
        # DMAs at packet granularity, so without a gate pair0 and pair1
        # would finish together; a tiny guard DMA that reads pair0's tile
        # stalls the SP sequencer until pair0 fully lands, giving pair0 the
        # full HBM bandwidth first.
        f_tiles = []
        guard = singles.tile([1, 2, 8], fp8)
        for pr in range(NPAIR):
            f_t = fpool.tile([128, V, 2, 2 * T], fp8)
            nc.sync.dma_start(out=f_t[:, : V // 2], in_=feat[pr, :, : V // 2])
            nc.sync.dma_start(out=f_t[:, V // 2 :], in_=feat[pr, :, V // 2 :])
            if pr == 0:
                nc.sync.dma_start(out=guard, in_=f_t[0:1, 0:2, 0, 0:8])
            f_tiles.append(f_t)

        w_t = singles.tile([128, 2, WPAD], fp8)
        nc.scalar.dma_start(out=w_t, in_=wmat[:, :, :])
        a_bc = singles.tile([128, 2 * VV], bf16)
        nc.scalar.dma_start(out=a_bc, in_=amat[:, :])
        g_t = singles.tile([2 * V + 1, VV], bf16)
        nc.scalar.dma_start(out=g_t, in_=gmat[:, :])
        # persistent per-pair sT staging tiles; row 36 is the constant ones
        # row that picks up G's c0 row in the score matmul
        sT_stage = [
            singles.tile([2 * V + 1, 2 * T], bf16, name=f"sT_stage{i}")
            for i in range(NPAIR)
        ]
        for s in sT_stage:
            # whole-tile memset (single-partition bases are not allowed);
            # rows 0..35 get overwritten by the per-pair PSUM copy, row 36
            # stays 1.0 as the bias row
            nc.gpsimd.memset(s, 1.0)

        # PE warm-up: junk matmuls from preamble-end until the first feat
        # tile lands keep the HAM activity window busy, so real matmuls run
        # at 2.4 GHz instead of the cold 1.2 GHz.  The HAM watches actual
        # array activity, so the junk data must toggle bits - iota, not
        # zeros.
        jsrc = singles.tile([128, 128], mybir.dt.int32)
        nc.gpsimd.iota(
            jsrc, pattern=[[1, 128]], base=7, channel_multiplier=97
        )
        jsrc16 = jsrc.bitcast(bf16)
        for wi in range(28):
            jp = ps_junk.tile([128, 128], f32)
            nc.tensor.matmul(
                out=jp,
                lhsT=jsrc16[:, 0:128],
                rhs=jsrc16[:, 0:128],
                start=True,
                stop=True,
            )

        def emit_pair(pr):
            f_t = f_tiles[pr]
            # --- TensorE: sT[(v,o), (b2 t)] over both batches of the pair ---
            sT_ps = ps_sT.tile([2 * V, 2 * T], f32)
            for v0 in range(V):
                nc.tensor.matmul(
                    out=sT_ps[:, :],
                    lhsT=w_t[:, :, 36 - 2 * v0 : 72 - 2 * v0],
                    rhs=f_t[:, v0, :, :],
                    start=(v0 == 0),
                    stop=(v0 == V - 1),
                    perf_mode=mybir.MatmulPerfMode.DoubleRow,
                )
            # --- ACT: evacuate to SBUF bf16 (rows 0..35 of the staged tile;
            # row 36 is the pre-set ones row) ---
            sT_sb = sT_stage[pr]
            nc.scalar.copy(out=sT_sb[: 2 * V, :], in_=sT_ps)
            return sT_sb

        def emit_batch(sT_sb, b, half, split):
            st = {"b": b}
            # --- TensorE: score[t, (i,j)] = sT.T @ G per t-block ---
            sc_ps = ps_sc.tile([128, 2, 512], f32)
            for tb in range(2):
                nc.tensor.matmul(
                    out=sc_ps[:, tb, :VV],
                    lhsT=sT_sb[
                        :, (2 * half + tb) * 128 : (2 * half + tb + 1) * 128
                    ],
                    rhs=g_t[:, :],
                    start=True,
                    stop=True,
                )
            # per-t-block tail units: each unit's chain is independent so
            # ACT/DVE/gpsimd pipeline across units
            st["units"] = []
            for tb in range(2):
                e1 = work.tile([128, VV], bf16)
                nc.scalar.activation(
                    out=e1, in_=sc_ps[:, tb, :VV],
                    func=mybir.ActivationFunctionType.Exp,
                    scale=1.0 / SC,
                )
                # negative branch: exp(.1 x / SC) with |arg| <= .07 -> the
                # linear Taylor 1 + .1x/SC is exact to ~2e-3 (< bf16 eps).
                # DVE is the busiest tail engine, so alternate units push
                # this op to ACT (Copy applies in*scale + bias there).
                e2 = work.tile([128, VV], bf16)
                if tb == 0:
                    nc.vector.tensor_scalar(
                        out=e2,
                        in0=sc_ps[:, tb, :VV],
                        scalar1=0.1 / SC,
                        scalar2=1.0,
                        op0=AluOpType.mult,
                        op1=AluOpType.add,
                    )
                else:
                    nc.scalar.activation(
                        out=e2,
                        in_=sc_ps[:, tb, :VV],
                        func=mybir.ActivationFunctionType.Copy,
                        bias=1.0,
                        scale=0.1 / SC,
                    )
                ex = work.tile([128, VV], bf16)
                nc.vector.tensor_tensor(
                    out=ex, in0=e1, in1=e2, op=AluOpType.max
                )
                # gpsimd: exa = ex * A
                exa = work.tile([128, VV], bf16)
                nc.gpsimd.tensor_mul(
                    out=exa, in0=ex,
                    in1=a_bc.rearrange("p (tb x) -> p tb x", tb=2)[:, tb],
                )
                # DVE: row-sum + reciprocal
                ssum = work.tile([128, V], f32)
                nc.vector.reduce_sum(
                    out=ssum,
                    in_=exa.rearrange("p (g j) -> p g j", j=V),
                    axis=mybir.AxisListType.X,
                )
                rec = work.tile([128, V], f32)
                nc.vector.reciprocal(out=rec, in_=ssum)
                att = opool.tile([128, VV], bf16)
                eng = nc.vector if (tb == 1 and b >= 2) else nc.gpsimd
                eng.tensor_mul(
                    out=att.rearrange("p (g j) -> p g j", j=V),
                    in0=exa.rearrange("p (g j) -> p g j", j=V),
                    in1=rec.unsqueeze(2).broadcast_to([128, V, V]),
                )
                st["units"].append((tb, tb + 1, att))
            return st

        def emit_out(st):
            for lo, hi, att in st["units"]:
                nc.sync.dma_start(
                    out=out[st["b"], :, lo * VV : hi * VV], in_=att
                )

        stages = []
        for pr in range(NPAIR):
            sT_sb = emit_pair(pr)
            for half in range(2):
                b = 2 * pr + half
                stages.append(emit_batch(sT_sb, b, half, split=True))
                if b >= 1:
                    emit_out(stages[b - 1])
        emit_out(stages[BPC - 1])
    return nc


def _prep_params(Wf, bf, Wa, ba):
    import ml_dtypes

    f8 = ml_dtypes.float8_e4m3fn
    bf16 = ml_dtypes.bfloat16
    w1, w2 = Wa[:64].astype(np.float64), Wa[64:].astype(np.float64)
    Wf64, bf64 = Wf.astype(np.float64), bf.astype(np.float64)
    u1 = (w1 @ Wf64) * SC
    u2 = (w2 @ Wf64) * SC
    c0 = float(w1 @ bf64 + w2 @ bf64 + float(ba[0]))
    wmat = np.zeros((128, 2, WPAD), dtype=f8)
    # u[kt*128 + p] at padded col 36 (u1) / 37 (u2)
    wmat[:, 0, 36] = u1[:128].astype(np.float32).astype(f8)
    wmat[:, 1, 36] = u1[128:].astype(np.float32).astype(f8)
    wmat[:, 0, 37] = u2[:128].astype(np.float32).astype(f8)
    wmat[:, 1, 37] = u2[128:].astype(np.float32).astype(f8)
    # G[(v,o), (i,j)]: score = s1[i] + s2[j] as a linear map of sT rows;
    # last row = SC*c0 against the constant ones row of sT_aug
    G = np.zeros((2 * V + 1, VV), dtype=np.float32)
    for v in range(V):
        G[2 * v + 0, v * V : (v + 1) * V] = 1.0  # s1[v] -> rows i == v
        G[2 * v + 1, v::V] = 1.0  # s2[v] -> cols j == v
    G[2 * V, :] = SC * c0
    gmat = G.astype(bf16)
    return wmat, gmat


def get_nc():
    global _cached_nc
    if _cached_nc is None:
        _cached_nc = _build_nc()
    return _cached_nc


def kernel(feat, A, Wf, bf, Wa, ba):
    _install_wait_legalizer()
    from concourse.bass_utils import run_bass_kernel_spmd

    import ml_dtypes

    f8 = ml_dtypes.float8_e4m3fn
    bf16 = ml_dtypes.bfloat16

    # [B, 256c, T, V] -> fp8, c=(kt,p); pairs of batches share one tile:
    # [pair, p, v, kt, (b2 t)]
    featq = np.asarray(feat, dtype=np.float32).astype(f8)
    featq = featq.reshape(B // 2, 2, 2, 128, T, V).transpose(0, 3, 5, 2, 1, 4)
    featq = np.ascontiguousarray(featq).reshape(B // 2, 128, V, 2, 2 * T)

    A2 = np.tile(np.asarray(A, np.float32).reshape(VV), 2).astype(bf16)
    A2 = np.broadcast_to(A2, (128, 2 * VV)).copy()
    wmat, gmat = _prep_params(
        np.asarray(Wf, np.float32),
        np.asarray(bf, np.float32),
        np.asarray(Wa, np.float32),
        np.asarray(ba, np.float32),
    )

    nc = get_nc()
    in_maps = [
        {
            "feat": featq[i * NPAIR : (i + 1) * NPAIR],
            "wmat": wmat,
            "amat": A2,
            "gmat": gmat,
        }
        for i in range(NCORES)
    ]
    # The first execution after a NEFF load can race on stale device
    # semaphore state left by other executables; the program's own epilogue
    # resets every semaphore, so discard one execution on the first call
    # and return the (consistent) second one.
    global _warmed
    if not _warmed:
        run_bass_kernel_spmd(nc, in_maps, core_ids=list(range(NCORES)))
        _warmed = True
    res = run_bass_kernel_spmd(nc, in_maps, core_ids=list(range(NCORES)))
    # out[b, p, (tb, i, j)] bf16 -> [b, t=(tb,p), i, j] f32
    outs = []
    for r in res.results:
        o = r["out"].astype(np.float32).reshape(BPC, 128, 2, V, V)
        outs.append(o.transpose(0, 2, 1, 3, 4).reshape(BPC, T, V, V))
    return np.concatenate(outs, axis=0)


# revision 31
# speedup vs baseline: 1.0527x; 1.0527x over previous
"""Trainium2 Bass kernel for nn_AttentionGenerator (gnn_message_passing).

Reference math:
    f = einsum('oc,bctv->botv', Wf, feat) + bf          # 1x1 conv, Cout=64
    s_i = einsum('c,bctv->btv', Wa[:64], f)
    s_j = einsum('c,bctv->btv', Wa[64:], f)
    score[b,t,i,j] = s_i[b,t,i] + s_j[b,t,j] + ba
    atten = (exp(leaky_relu(score)) * A) / row_sum

Because f only enters through the two dot products, fold Wf/bf/Wa/ba on
the host into u1 = w1@Wf, u2 = w2@Wf (length-256 vectors) and the scalar
c0 = (w1+w2)@bf + ba.

Device pipeline (memory-bound problem -> minimize HBM bytes):
  * feat is sent in fp8 e4m3 (halves the dominant HBM stream vs bf16);
    u1/u2 are scaled by 64 into the fp8 normal range, with the 1/64
    descale folded into the exp input scale (max/mult commute with
    positive scaling, so leaky-relu can run on the scaled scores).
  * TensorE pass 1: per batch-pair, 18 DoubleRow matmuls (2 fp8
    contraction rows/cycle) contract the 256 channels for each graph
    node v -> sT[(v,o), t] in PSUM.  The 36 stationary columns are
    u1/u2 shifted per-v, so s_i and s_j come from a single feat pass.
  * ACT evacuates sT to a staged SBUF tile (bf16) whose row 36 is a
    constant ones row.
  * TensorE pass 2: score[t, (i,j)] = s1[t,i]+s2[t,j]+c0 is linear in
    the augmented sT, so one tiny matmul per t-block against a constant
    matrix G [37, 324] (0/1 pattern + c0 row) assembles the full biased
    score tile - no transposes, no DMA scatters, no broadcast-add.
  * exp(leaky(x)) = max(exp(x), exp(.1x)), and exp(.1x/64) with
    |arg|<=.07 is the linear Taylor 1+.1x/64 to ~2e-3: e1 = Exp on ACT
    runs in parallel with the e2 Taylor on DVE (single Exp table,
    loaded once - table switches cost 1.5us), then a packed-bf16 max.
  * gpsimd: *A; DVE: row-sum + reciprocal; normalize alternates
    DVE/gpsimd per t-block so both halves finish in parallel.
  * Junk matmuls on iota data during the initial feat DMA warm the PE
    HAM clock gate (cold PE runs at 1.2 GHz, warm 2.4; the monitor
    watches actual bit activity, so the data must be nonzero).
  * A guard DMA gates pair1's feat behind pair0 (the HWDGE rings
    round-robin all queued DMAs, which would starve pair0 otherwise).
  * Outputs are written bf16 and upcast on the host (atten in [0,1]).

Sharding: pure data parallel - batch B=32 split across 8 NeuronCores
(4 batches each), tiny params replicated, no cross-core comms.
"""

import json
import numpy as np
from contextlib import ExitStack

B, Cin, T, V = 32, 256, 256, 18
NCORES = 8
BPC = B // NCORES  # batches per core
NPAIR = BPC // 2  # batch pairs per core
SC = 64.0  # weight prescale so u1/u2 land in fp8-normal range
VV = V * V  # 324
WPAD = 80  # padded weight column pitch (DoubleRow k-tile step must be %16)

_cached_nc = None
_warmed = False


def _legalize_waits_json(bir_json):
    """Split instructions carrying >1 sync wait into single-wait NoOps plus
    the original instruction.  The walrus build in this container accepts at
    most ONE sync-wait command per instruction struct; concourse's Tile
    scheduler freely attaches several.  Hoisting the extra waits onto NoOps
    immediately before the instruction (same engine stream, same position)
    preserves semantics exactly - engines execute their stream in order."""
    bir = json.loads(bir_json)
    ctr = 0
    for fn in bir.get("functions", []):
        for blk in fn.get("blocks", []):
            insts = blk.get("instructions")
            if not insts:
                continue
            out = []
            for inst in insts:
                si = inst.get("sync_info") or {}
                waits = si.get("on_wait") or []
                if len(waits) > 1:
                    for w in waits[:-1]:
                        out.append(
                            {
                                "engine": inst.get("engine"),
                                "ins": [],
                                "name": f"wsplit-{ctr}",
                                "opcode": "NoOp",
                                "outs": [],
                                "sync_info": {"on_update": [], "on_wait": [w]},
                            }
                        )
                        ctr += 1
                    si = dict(si)
                    si["on_wait"] = [waits[-1]]
                    inst = dict(inst)
                    inst["sync_info"] = si
                out.append(inst)
            blk["instructions"] = out
    return json.dumps(bir).encode()


_wait_patch_done = False


def _install_wait_legalizer():
    global _wait_patch_done
    if _wait_patch_done:
        return
    import concourse.bass_utils as bass_utils
    import concourse.bass2jax as bass2jax

    orig = bass_utils.compile_bir_kernel

    def wrapped(bir_json, tmpdir, neff_name="file.neff"):
        return orig(_legalize_waits_json(bir_json), tmpdir, neff_name)

    bass_utils.compile_bir_kernel = wrapped
    bass2jax.compile_bir_kernel = wrapped
    _wait_patch_done = True


def _build_nc():
    import concourse.bass as bass
    import concourse.mybir as mybir
    import concourse.tile as tile
    from concourse.alu_op_type import AluOpType

    f32 = mybir.dt.float32
    bf16 = mybir.dt.bfloat16
    fp8 = mybir.dt.float8e4
    nc = bass.Bass(num_swdge_queues=4)

    # feat[pair, p, v, kt, (b2 t)]: channel c = kt*128 + p, fp8 e4m3
    feat = nc.dram_tensor(
        "feat", [NPAIR, 128, V, 2, 2 * T], fp8, kind="ExternalInput"
    )
    # wmat[p, kt, col]: zeros except col 36 = 64*u1[kt*128+p], col 37 = 64*u2
    wmat = nc.dram_tensor("wmat", [128, 2, WPAD], fp8, kind="ExternalInput")
    # amat[p, (tb, i, j)]: A duplicated per t-block, pre-replicated across
    # partitions on the host (a partition-broadcast DMA hammers one DRAM
    # page and steals SDMA bandwidth from the feat stream for ~10us)
    amat = nc.dram_tensor("amat", [128, 2 * VV], bf16, kind="ExternalInput")
    # gmat[(v,o) + ones-row, (i,j)]: score-assembly 0/1 matrix with the
    # bias row SC*c0 last - score = sT_aug.T @ G lands fully biased
    gmat = nc.dram_tensor("gmat", [2 * V + 1, VV], bf16, kind="ExternalInput")
    out = nc.dram_tensor("out", [BPC, 128, 2 * VV], bf16, kind="ExternalOutput")

    with ExitStack() as ctx:
        tc = ctx.enter_context(tile.TileContext(nc))
        singles = ctx.enter_context(tc.tile_pool(name="singles", bufs=1))
        fpool = ctx.enter_context(tc.tile_pool(name="fpool", bufs=NPAIR))
        ps_sT = ctx.enter_context(tc.tile_pool(name="ps_sT", bufs=2, space="PSUM"))
        ps_sc = ctx.enter_context(tc.tile_pool(name="ps_sc", bufs=2, space="PSUM"))
        ps_junk = ctx.enter_context(tc.tile_pool(name="ps_junk", bufs=2, space="PSUM"))
        work = ctx.enter_context(tc.tile_pool(name="work", bufs=2))
        opool = ctx.enter_context(tc.tile_pool(name="opool", bufs=2))

        # feat loads first (highest tile priority): 2 per pair (v 0:9 /
        # 9:18) on the SP ring.  The HWDGE rings round-robin ALL queued
        # DMAs at packet granularity, so without a gate pair0 and pair1
        # would finish together; a tiny guard DMA that reads pair0's tile
        # stalls the SP sequencer until pair0 fully lands, giving pair0 the
        # full HBM bandwidth first.
        f_tiles = []
        guard = singles.tile([1, 2, 8], fp8)
        for pr in range(NPAIR):
            f_t = fpool.tile([128, V, 2, 2 * T], fp8)
            nc.sync.dma_start(out=f_t[:, : V // 2], in_=feat[pr, :, : V // 2])
            nc.sync.dma_start(out=f_t[:, V // 2 :], in_=feat[pr, :, V // 2 :])
            if pr == 0:
                nc.sync.dma_start(out=guard, in_=f_t[0:1, 0:2, 0, 0:8])
            f_tiles.append(f_t)

        w_t = singles.tile([128, 2, WPAD], fp8)
        nc.scalar.dma_start(out=w_t, in_=wmat[:, :, :])
        a_bc = singles.tile([128, 2 * VV], bf16)
        nc.scalar.dma_start(out=a_bc, in_=amat[:, :])
        g_t = singles.tile([2 * V + 1, VV], bf16)
        nc.scalar.dma_start(out=g_t, in_=gmat[:, :])
        # persistent per-pair sT staging tiles; row 36 is the constant ones
        # row that picks up G's c0 row in the score matmul
        sT_stage = [
            singles.tile([2 * V + 1, 2 * T], bf16, name=f"sT_stage{i}")
            for i in range(NPAIR)
        ]
        for s in sT_stage:
            # whole-tile memset (single-partition bases are not allowed);
            # rows 0..35 get overwritten by the per-pair PSUM copy, row 36
            # stays 1.0 as the bias row
            nc.gpsimd.memset(s, 1.0)

        # PE warm-up: junk matmuls from preamble-end until the first feat
        # tile lands keep the HAM activity window busy, so real matmuls run
        # at 2.4 GHz instead of the cold 1.2 GHz.  The HAM watches actual
        # array activity, so the junk data must toggle bits - iota, not
        # zeros.
        jsrc = singles.tile([128, 128], mybir.dt.int32)
        nc.gpsimd.iota(
            jsrc, pattern=[[1, 128]], base=7, channel_multiplier=97
        )
        jsrc16 = jsrc.bitcast(bf16)
        for wi in range(28):
            jp = ps_junk.tile([128, 128], f32)
            nc.tensor.matmul(
                out=jp,
                lhsT=jsrc16[:, 0:128],
                rhs=jsrc16[:, 0:128],
                start=True,
                stop=True,
            )

        def emit_pair(pr):
            f_t = f_tiles[pr]
            # --- TensorE: sT[(v,o), (b2 t)] over both batches of the pair ---
            sT_ps = ps_sT.tile([2 * V, 2 * T], f32)
            for v0 in range(V):
                nc.tensor.matmul(
                    out=sT_ps[:, :],
                    lhsT=w_t[:, :, 36 - 2 * v0 : 72 - 2 * v0],
                    rhs=f_t[:, v0, :, :],
                    start=(v0 == 0),
                    stop=(v0 == V - 1),
                    perf_mode=mybir.MatmulPerfMode.DoubleRow,
                )
            # --- ACT: evacuate to SBUF bf16 (rows 0..35 of the staged tile;
            # row 36 is the pre-set ones row) ---
            sT_sb = sT_stage[pr]
            nc.scalar.copy(out=sT_sb[: 2 * V, :], in_=sT_ps)
            return sT_sb

        def emit_batch(sT_sb, b, half, split):
            st = {"b": b}
            # --- TensorE: score[t, (i,j)] = sT.T @ G per t-block ---
            sc_ps = ps_sc.tile([128, 2, 512], f32)
            for tb in range(2):
                nc.tensor.matmul(
                    out=sc_ps[:, tb, :VV],
                    lhsT=sT_sb[
                        :, (2 * half + tb) * 128 : (2 * half + tb + 1) * 128
                    ],
                    rhs=g_t[:, :],
                    start=True,
                    stop=True,
                )
            # per-t-block tail units: each unit's chain is independent so
            # ACT/DVE/gpsimd pipeline across units
            st["units"] = []
            for tb in range(2):
                e1 = work.tile([128, VV], bf16)
                nc.scalar.activation(
                    out=e1, in_=sc_ps[:, tb, :VV],
                    func=mybir.ActivationFunctionType.Exp,
                    scale=1.0 / SC,
                )
                # negative branch: exp(.1 x / SC) with |arg| <= .07 -> the
                # linear Taylor 1 + .1x/SC is exact to ~2e-3 (< bf16 eps);
                # runs on DVE in parallel with e1 on ACT (the tail is
                # chain-latency-bound, so e1 and e2 must not share an engine)
                e2 = work.tile([128, VV], bf16)
                nc.vector.tensor_scalar(
                    out=e2,
                    in0=sc_ps[:, tb, :VV],
                    scalar1=0.1 / SC,
                    scalar2=1.0,
                    op0=AluOpType.mult,
                    op1=AluOpType.add,
                )
                ex = work.tile([128, VV], bf16)
                nc.vector.tensor_tensor(
                    out=ex, in0=e1, in1=e2, op=AluOpType.max
                )
                # gpsimd: exa = ex * A
                exa = work.tile([128, VV], bf16)
                nc.gpsimd.tensor_mul(
                    out=exa, in0=ex,
                    in1=a_bc.rearrange("p (tb x) -> p tb x", tb=2)[:, tb],
                )
                # DVE: row-sum + reciprocal
                ssum = work.tile([128, V], f32)
                nc.vector.reduce_sum(
                    out=ssum,
                    in_=exa.rearrange("p (g j) -> p g j", j=V),
                    axis=mybir.AxisListType.X,
                )
                rec = work.tile([128, V], f32)
                nc.vector.reciprocal(out=rec, in_=ssum)
                att = opool.tile([128, VV], bf16)
                eng = nc.vector if tb == 1 else nc.gpsimd
                eng.tensor_mul(
                    out=att.rearrange("p (g j) -> p g j", j=V),
                    in0=exa.rearrange("p (g j) -> p g j", j=V),
                    in1=rec.unsqueeze(2).broadcast_to([128, V, V]),
                )
                st["units"].append((tb, tb + 1, att))
            return st

        def emit_out(st):
            for lo, hi, att in st["units"]:
                nc.sync.dma_start(
                    out=out[st["b"], :, lo * VV : hi * VV], in_=att
                )

        stages = []
        for pr in range(NPAIR):
            sT_sb = emit_pair(pr)
            for half in range(2):
                b = 2 * pr + half
                stages.append(emit_batch(sT_sb, b, half, split=True))
                if b >= 1:
                    emit_out(stages[b - 1])
        emit_out(stages[BPC - 1])
    return nc


def _prep_params(Wf, bf, Wa, ba):
    import ml_dtypes

    f8 = ml_dtypes.float8_e4m3fn
    bf16 = ml_dtypes.bfloat16
    w1, w2 = Wa[:64].astype(np.float64), Wa[64:].astype(np.float64)
    Wf64, bf64 = Wf.astype(np.float64), bf.astype(np.float64)
    u1 = (w1 @ Wf64) * SC
    u2 = (w2 @ Wf64) * SC
    c0 = float(w1 @ bf64 + w2 @ bf64 + float(ba[0]))
    wmat = np.zeros((128, 2, WPAD), dtype=f8)
    # u[kt*128 + p] at padded col 36 (u1) / 37 (u2)
    wmat[:, 0, 36] = u1[:128].astype(np.float32).astype(f8)
    wmat[:, 1, 36] = u1[128:].astype(np.float32).astype(f8)
    wmat[:, 0, 37] = u2[:128].astype(np.float32).astype(f8)
    wmat[:, 1, 37] = u2[128:].astype(np.float32).astype(f8)
    # G[(v,o), (i,j)]: score = s1[i] + s2[j] as a linear map of sT rows;
    # last row = SC*c0 against the constant ones row of sT_aug
    G = np.zeros((2 * V + 1, VV), dtype=np.float32)
    for v in range(V):
        G[2 * v + 0, v * V : (v + 1) * V] = 1.0  # s1[v] -> rows i == v
        G[2 * v + 1, v::V] = 1.0  # s2[v] -> cols j == v
    G[2 * V, :] = SC * c0
    gmat = G.astype(bf16)
    return wmat, gmat


def get_nc():
    global _cached_nc
    if _cached_nc is None:
        _cached_nc = _build_nc()
    return _cached_nc


def kernel(feat, A, Wf, bf, Wa, ba):
    _install_wait_legalizer()
    from concourse.bass_utils import run_bass_kernel_spmd

    import ml_dtypes

    f8 = ml_dtypes.float8_e4m3fn
    bf16 = ml_dtypes.bfloat16

    # [B, 256c, T, V] -> fp8, c=(kt,p); pairs of batches share one tile:
    # [pair, p, v, kt, (b2 t)]
    featq = np.asarray(feat, dtype=np.float32).astype(f8)
    featq = featq.reshape(B // 2, 2, 2, 128, T, V).transpose(0, 3, 5, 2, 1, 4)
    featq = np.ascontiguousarray(featq).reshape(B // 2, 128, V, 2, 2 * T)

    A2 = np.tile(np.asarray(A, np.float32).reshape(VV), 2).astype(bf16)
    A2 = np.broadcast_to(A2, (128, 2 * VV)).copy()
    wmat, gmat = _prep_params(
        np.asarray(Wf, np.float32),
        np.asarray(bf, np.float32),
        np.asarray(Wa, np.float32),
        np.asarray(ba, np.float32),
    )

    nc = get_nc()
    in_maps = [
        {
            "feat": featq[i * NPAIR : (i + 1) * NPAIR],
            "wmat": wmat,
            "amat": A2,
            "gmat": gmat,
        }
        for i in range(NCORES)
    ]
    # The first execution after a NEFF load can race on stale device
    # semaphore state left by other executables; the program's own epilogue
    # resets every semaphore, so discard one execution on the first call
    # and return the (consistent) second one.
    global _warmed
    if not _warmed:
        run_bass_kernel_spmd(nc, in_maps, core_ids=list(range(NCORES)))
        _warmed = True
    res = run_bass_kernel_spmd(nc, in_maps, core_ids=list(range(NCORES)))
    # out[b, p, (tb, i, j)] bf16 -> [b, t=(tb,p), i, j] f32
    outs = []
    for r in res.results:
        o = r["out"].astype(np.float32).reshape(BPC, 128, 2, V, V)
        outs.append(o.transpose(0, 2, 1, 3, 4).reshape(BPC, T, V, V))
    return np.concatenate(outs, axis=0)
